# revision 1
# baseline (speedup 1.0000x reference)
import sys
sys.path.insert(0, "/opt/trn_rl_repo")
"""Host-side preparation: sharding, sorting, padding, weight preprocessing.

Layout conventions (P=128):
- Phase A (TP) slot id: s = p*T_A + t  (partition-major). Per-slot arrays are
  uploaded as [P, T_A] with [p, t] <-> slot p*T_A + t. Tile t = column t.
  Tiles are bucket-pure: bucket b owns tile columns [TAoff[b], TAoff[b+1]).
- act scratch table in DRAM: [P*T_A, 32], row s.
- Phase B (scatter) slot id: s' = t*P + p. Arrays [P, T_B] with [p, t] <->
  slot t*P+p. Tiles are dst-block-pure: block kb owns columns
  [TKoff[kb], TKoff[kb+1]). d_local in [0,128) or 255 (dead).
- piB [P, T_B] int32: A-slot gathered for each B-slot (0 for dead).
"""
import numpy as np

MAX_Z = 100
DIM = 32
STEP = 0.25
N_MU = 21
N_SCAL = N_MU + 1
N_LAYERS = 3
NB = 20
P = 128
D = 4  # polynomial degree (channels per edge = D*DIM = 128)


def poly_coeffs():
    mu = np.linspace(0.0, 5.0, N_MU)
    C = np.zeros((NB, D, N_SCAL))
    for b in range(NB):
        rr = np.linspace(b * STEP, (b + 1) * STEP, 257)
        u = (rr - b * STEP) / STEP * 2.0 - 1.0
        V = np.stack([u ** d for d in range(D)], 1)
        G = np.exp(-0.5 * ((rr[:, None] - mu[None, :]) / STEP) ** 2)
        G = np.concatenate([G, np.ones((len(rr), 1))], 1)
        C[b], *_ = np.linalg.lstsq(V, G, rcond=None)
    return C  # [NB, D, N_SCAL]


def build_wb_all(C, w_s2d, w_d2s, scale):
    """Wb for every (layer, dir, bucket): [L, 2, NB, D*DIM, DIM] f32."""
    L = w_s2d.shape[0]
    out = np.zeros((L, 2, NB, D * DIM, DIM), np.float32)
    for l in range(L):
        out[l, 0] = np.einsum("bdi,ijk->bdjk", C, w_s2d[l]).reshape(NB, D * DIM, DIM) * scale
        out[l, 1] = np.einsum("bdi,ijk->bdjk", C, w_d2s[l]).reshape(NB, D * DIM, DIM) * scale
    return out


def _ceil_mul(x, m):
    return (x + m - 1) // m * m


def shard_direction(owner, other, r, z_other, n_cores, npc, gbatch):
    """owner: node owning each edge in this direction (scatter target).
       other: the gathered-side node index. z_other: z of gathered side.
       Returns per-core arrays + meta with static tile counts."""
    E = len(owner)
    core_of = owner // npc
    bucket = np.clip((r / STEP).astype(np.int64), 0, NB - 1)
    n_blocks = (npc + P - 1) // P

    idx_by_core = [np.nonzero(core_of == c)[0] for c in range(n_cores)]

    TB = np.zeros(NB, np.int64)
    for c in range(n_cores):
        cnt = np.bincount(bucket[idx_by_core[c]], minlength=NB)
        TB = np.maximum(TB, (cnt + P - 1) // P)
    live_A = int(TB.sum())
    T_A = _ceil_mul(max(live_A, 1), gbatch)
    TB[-1] += T_A - int(TB.sum())  # absorb rounding tiles into last bucket
    TAoff = np.concatenate([[0], np.cumsum(TB)]).astype(np.int64)

    TK = np.zeros(n_blocks, np.int64)
    for c in range(n_cores):
        loc = owner[idx_by_core[c]] - c * npc
        cnt = np.bincount(loc // P, minlength=n_blocks)
        TK = np.maximum(TK, (cnt + P - 1) // P)
    live_B = int(TK.sum())
    T_B = _ceil_mul(max(live_B, 1), gbatch)
    TK[-1] += T_B - int(TK.sum())
    TKoff = np.concatenate([[0], np.cumsum(TK)]).astype(np.int64)

    cores = []
    for c in range(n_cores):
        idx = idx_by_core[c]
        bc = bucket[idx]
        rA = np.zeros((P, T_A), np.float32)
        vA = np.zeros((P, T_A), np.float32)
        gidx_node = np.zeros((P, T_A), np.int32)
        gidx_emb = np.zeros((P, T_A), np.int32)
        slotA = np.zeros(len(idx), np.int64)
        for b in range(NB):
            rA[:, TAoff[b]:TAoff[b + 1]] = (b + 0.5) * STEP
            sel = np.nonzero(bc == b)[0]
            if len(sel) == 0:
                continue
            # fill column-major within the bucket's tile range: local slot
            # q -> tile TAoff[b] + q // P, partition q % P
            q = np.arange(len(sel))
            tcol = TAoff[b] + q // P
            prow = q % P
            slotA[sel] = prow * T_A + tcol
            rA[prow, tcol] = r[idx[sel]]
            vA[prow, tcol] = 1.0
            gidx_node[prow, tcol] = other[idx[sel]]
            gidx_emb[prow, tcol] = z_other[other[idx[sel]]]
        # phase B
        loc = owner[idx] - c * npc
        blk = loc // P
        dloc = np.full((P, T_B), 255.0, np.float32)
        piB = np.zeros((P, T_B), np.int32)
        for kb in range(n_blocks):
            sel = np.nonzero(blk == kb)[0]
            if len(sel) == 0:
                continue
            q = np.arange(len(sel))
            tcol = TKoff[kb] + q // P
            prow = q % P
            dloc[prow, tcol] = (loc[sel] % P).astype(np.float32)
            piB[prow, tcol] = slotA[sel]
        cores.append(dict(rA=rA, vA=vA, gidx_node=gidx_node, gidx_emb=gidx_emb,
                          dloc=dloc, piB=piB))

    meta = dict(TB=TB, TAoff=TAoff, T_A=T_A, TK=TK, TKoff=TKoff, T_B=T_B,
                n_blocks=n_blocks, live_A=live_A, live_B=live_B)
    return cores, meta


def prepare(inputs, n_cores=8, gbatch=64):
    z0 = np.asarray(inputs["z0"]).astype(np.int64)
    z1 = np.asarray(inputs["z1"]).astype(np.int64)
    src = np.asarray(inputs["src"]).astype(np.int64)
    dst = np.asarray(inputs["dst"]).astype(np.int64)
    r = np.asarray(inputs["r"], np.float64)
    emb_w = np.asarray(inputs["emb_w"], np.float32)
    emb_b = np.asarray(inputs["emb_b"], np.float32)
    w_s2d = np.asarray(inputs["w_s2d"], np.float32)
    w_d2s = np.asarray(inputs["w_d2s"], np.float32)
    ro_w = np.asarray(inputs["ro_w"], np.float32)
    ro_b = np.asarray(inputs["ro_b"], np.float32)
    N0, N1 = len(z0), len(z1)
    npc0, npc1 = N0 // n_cores, N1 // n_cores
    assert npc0 * n_cores == N0 and npc1 * n_cores == N1

    C = poly_coeffs()
    scale = 1.0 / np.sqrt(N_SCAL * DIM) / np.sqrt(N0 + N1)
    wb_all = build_wb_all(C, w_s2d, w_d2s, scale)  # [3,2,NB,128,32]

    emb_eff = (emb_w + emb_b[None, :]).astype(np.float32)  # [100, 32]

    # dir 0 (s2d): owner=dst (scatter to y1), gather side = src nodes (y0/z0)
    s2d_cores, s2d_meta = shard_direction(dst, src, r, z0, n_cores, npc1, gbatch)
    # dir 1 (d2s): owner=src (scatter to y0), gather side = dst nodes (y1/z1)
    d2s_cores, d2s_meta = shard_direction(src, dst, r, z1, n_cores, npc0, gbatch)

    # per-core z slices for layer-0 y_old slice gathers (node-major [P, nblk] layout:
    # node n = c*npc + kb*P + p  -> column kb, partition p)
    def z_slice(z, npc, c):
        nblk = (npc + P - 1) // P
        zz = np.zeros(nblk * P, z.dtype)
        zz[:npc] = z[c * npc:(c + 1) * npc]
        return zz.reshape(nblk, P).T.astype(np.int32).copy()

    per_core = []
    for c in range(n_cores):
        d = dict()
        for k, v in s2d_cores[c].items():
            d[k + "_0"] = v
        for k, v in d2s_cores[c].items():
            d[k + "_1"] = v
        d["zsl_0"] = z_slice(z1, npc1, c)  # dir0 updates y1 slice
        d["zsl_1"] = z_slice(z0, npc0, c)  # dir1 updates y0 slice
        d["wb_all"] = wb_all.reshape(-1, DIM)  # same on all cores
        d["emb_eff"] = emb_eff
        d["ro_w"] = ro_w
        per_core.append(d)

    meta = dict(s2d=s2d_meta, d2s=d2s_meta, n_cores=n_cores,
                npc0=npc0, npc1=npc1, ro_b=float(ro_b[0]), gbatch=gbatch,
                TB_s2d=s2d_meta["TB"], TB_d2s=d2s_meta["TB"])
    return per_core, meta


"""Bass/Tile kernel builder for the dimer GNN. See prep.py for layouts."""
from contextlib import ExitStack

import numpy as np
from concourse import bass, mybir
from concourse import bacc
import concourse.tile as tile
from concourse.masks import make_identity

P = 128
DIM = 32
NB = 20
D = 4
FP = mybir.dt.float32
BF = mybir.dt.bfloat16
I32 = mybir.dt.int32
AF = mybir.ActivationFunctionType
ALU = mybir.AluOpType


def build_program(meta, shapes, edge_dtype=FP, n_layers=3):
    n_cores = meta["n_cores"]
    npc0, npc1 = meta["npc0"], meta["npc1"]
    G = meta["gbatch"]
    ED = edge_dtype

    nc = bacc.Bacc("TRN2", target_bir_lowering=False, debug=False,
                   num_devices=n_cores)

    ins = {}
    for name, shp in shapes.items():
        dt = I32 if name.startswith(("gidx", "piB", "zsl")) else FP
        ins[name] = nc.dram_tensor(name, list(shp), dt, kind="ExternalInput")

    out_t = nc.dram_tensor("out", [1, 1], FP, kind="ExternalOutput")

    TA = {0: meta["s2d"]["T_A"], 1: meta["d2s"]["T_A"]}
    act_dram = {d: nc.dram_tensor(f"act_scratch_{d}", [P * TA[d], DIM], ED)
                for d in (0, 1)}
    npc = {0: npc1, 1: npc0}          # dir -> owned-slice length
    side_of_dir = {0: 1, 1: 0}        # dir0 updates y1 (side1); dir1 -> y0
    gside_of_dir = {0: 0, 1: 1}       # dir0 gathers y0; dir1 gathers y1
    cc_in = {d: nc.dram_tensor(f"cc_in_{d}", [npc[d], DIM], FP) for d in (0, 1)}
    NT = {0: npc0 * n_cores, 1: npc1 * n_cores}
    y_tab = {}
    for side in (0, 1):
        for ph in range(n_layers - 1):
            y_tab[(side, ph)] = nc.dram_tensor(
                f"y_tab_{side}_{ph}", [NT[side], DIM], FP, addr_space="Shared")
    ar_in = nc.dram_tensor("ar_in", [1, 1], FP)
    ar_out = nc.dram_tensor("ar_out", [1, 1], FP, addr_space="Shared")

    replica_groups = [list(range(n_cores))]
    WBCOLS = n_layers * 2 * NB * DIM

    with tile.TileContext(nc) as tc, ExitStack() as ctx:
        const = ctx.enter_context(tc.tile_pool(name="const", bufs=1))
        identity = const.tile([P, P], FP)
        make_identity(nc, identity[:])
        iota_i = const.tile([P, P], I32)
        nc.gpsimd.iota(iota_i[:], [[1, P]], base=0, channel_multiplier=0)
        iota = const.tile([P, P], FP)
        nc.vector.tensor_copy(out=iota[:], in_=iota_i[:])
        ro_w_sb = const.tile([DIM, 1], FP)
        nc.sync.dma_start(ro_w_sb[:], ins["ro_w"][:])
        emb128 = const.tile([P, DIM], FP)
        nc.vector.memset(emb128[:], 0.0)
        nc.sync.dma_start(emb128[:100, :], ins["emb_eff"][:])
        wb_sb = const.tile([P, WBCOLS], FP)
        nc.sync.dma_start(
            wb_sb[:].rearrange("p (a k) -> p a k", k=DIM),
            ins["wb_all"][:].rearrange("(a p) k -> p a k", p=P))
        if ED != FP:
            wb_ed = const.tile([P, WBCOLS], ED)
            nc.vector.tensor_copy(out=wb_ed[:], in_=wb_sb[:])
        nblk_of = {d: (npc[d] + P - 1) // P for d in (0, 1)}
        yT = {d: const.tile([DIM, nblk_of[d] * P], FP, name=f"yT{d}",
                            tag=f"yT{d}")
              for d in (0, 1)}

        def wb_ap(l, d, b):
            off = ((l * 2 + d) * NB + b) * DIM
            src = wb_ed if ED != FP else wb_sb
            return src[:, off:off + DIM]

        # ---------------- node-state init: yT = emb_eff[z_slice].T ----------
        with ExitStack() as ictx:
            pi_ = ictx.enter_context(tc.tile_pool(name="init", bufs=2))
            psI = ictx.enter_context(
                tc.tile_pool(name="psI", bufs=2, space="PSUM"))
            for d in (0, 1):
                nblk = nblk_of[d]
                zsl = pi_.tile([P, nblk], I32, tag="zsl")
                nc.sync.dma_start(zsl[:], ins[f"zsl_{d}"][:])
                rows = pi_.tile([P, nblk * DIM], FP, tag="initrows")
                for kb2 in range(nblk):
                    nc.gpsimd.indirect_dma_start(
                        out=rows[:, kb2 * DIM:(kb2 + 1) * DIM],
                        out_offset=None, in_=ins["emb_eff"][:],
                        in_offset=bass.IndirectOffsetOnAxis(
                            ap=zsl[:, kb2:kb2 + 1], axis=0))
                for kb in range(nblk):
                    tp = psI.tile([DIM, P], FP, tag="initT")
                    nc.tensor.transpose(
                        out=tp[:], in_=rows[:, kb * DIM:(kb + 1) * DIM],
                        identity=identity[:])
                    nc.vector.tensor_copy(out=yT[d][:, kb * P:(kb + 1) * P],
                                          in_=tp[:])

        # ---------------- per-direction-layer emitter ----------------
        def emit_phaseA(l, d):
            m = meta["s2d"] if d == 0 else meta["d2s"]
            live_A = m["live_A"]
            T_A, T_B = m["T_A"], m["T_B"]
            TAoff, TKoff = m["TAoff"], m["TKoff"]
            sfx = f"_{d}"

            with ExitStack() as dlctx:
                io = dlctx.enter_context(tc.tile_pool(name=f"io{l}{d}", bufs=1))
                actx = dlctx.enter_context(ExitStack())
                pA = actx.enter_context(tc.tile_pool(name=f"pA{l}{d}", bufs=4))
                pXT = actx.enter_context(tc.tile_pool(name=f"pXT{l}{d}", bufs=8))
                psA = actx.enter_context(
                    tc.tile_pool(name=f"psA{l}{d}", bufs=2, space="PSUM"))
                psM = actx.enter_context(
                    tc.tile_pool(name=f"psM{l}{d}", bufs=2, space="PSUM"))

                r_sb = io.tile([P, T_A], FP)
                v_sb = io.tile([P, T_A], FP)
                gidx_sb = io.tile([P, T_A], I32)
                nc.sync.dma_start(r_sb[:], ins["rA" + sfx][:])
                nc.sync.dma_start(v_sb[:], ins["vA" + sfx][:])
                gname = ("gidx_emb" if l == 0 else "gidx_node") + sfx
                nc.sync.dma_start(gidx_sb[:], ins[gname][:])
                if l == 0:
                    zf_sb = io.tile([P, T_A], FP)
                    nc.vector.tensor_copy(out=zf_sb[:], in_=gidx_sb[:])

                u_sb = io.tile([P, T_A], FP)
                for b in range(NB):
                    c0, c1 = int(TAoff[b]), int(TAoff[b + 1])
                    if c1 > c0:
                        nc.vector.tensor_scalar(
                            out=u_sb[:, c0:c1], in0=r_sb[:, c0:c1],
                            scalar1=8.0, scalar2=float(2 * b + 1),
                            op0=ALU.mult, op1=ALU.subtract)
                pw_sb = io.tile([P, T_A * D], FP)
                pw3 = pw_sb[:].rearrange("p (t d) -> p t d", d=D)
                nc.vector.tensor_copy(out=pw3[:, :, 0], in_=v_sb[:])
                for dd in range(1, D):
                    nc.vector.tensor_tensor(
                        out=pw3[:, :, dd], in0=pw3[:, :, dd - 1], in1=u_sb[:],
                        op=ALU.mult)

                if l == 0:
                    gather_tab = ins["emb_eff"]
                else:
                    gather_tab = y_tab[(gside_of_dir[d], l - 1)]
                act_rows = act_dram[d][:].rearrange("(p t) k -> p t k", t=T_A)

                # ---------------- phase A ----------------
                for ga in range(T_A // G):
                    t0 = ga * G
                    nlive = max(0, min(G, live_A - t0))
                    atoms = pA.tile([P, G * DIM], FP, tag="atoms")
                    if l == 0:
                        for gg in range(nlive):
                            tt = t0 + gg
                            ohz = pXT.tile([P, P], FP, tag="ohz")
                            nc.vector.tensor_scalar(
                                out=ohz[:], in0=iota[:],
                                scalar1=zf_sb[:, tt:tt + 1], scalar2=None,
                                op0=ALU.is_equal)
                            ohzT_ps = psA.tile([P, P], FP, tag="ohzT")
                            nc.tensor.transpose(
                                out=ohzT_ps[:], in_=ohz[:],
                                identity=identity[:])
                            ohzT = pXT.tile([P, P], FP, tag="ohzTs")
                            if gg % 2 == 0:
                                nc.scalar.copy(out=ohzT[:], in_=ohzT_ps[:])
                            else:
                                nc.vector.tensor_copy(out=ohzT[:], in_=ohzT_ps[:])
                            at_ps = psM.tile([P, DIM], FP, tag="atps")
                            nc.tensor.matmul(
                                out=at_ps[:], lhsT=ohzT[:], rhs=emb128[:],
                                start=True, stop=True)
                            nc.scalar.copy(
                                out=atoms[:, gg * DIM:(gg + 1) * DIM],
                                in_=at_ps[:])
                    else:
                        for gg in range(nlive):
                            nc.gpsimd.indirect_dma_start(
                                out=atoms[:, gg * DIM:(gg + 1) * DIM],
                                out_offset=None, in_=gather_tab[:],
                                in_offset=bass.IndirectOffsetOnAxis(
                                    ap=gidx_sb[:, t0 + gg:t0 + gg + 1], axis=0))
                    act_slab = pA.tile([P, G * DIM], ED, tag="act")
                    if nlive < G:
                        nc.vector.memset(act_slab[:, nlive * DIM:], 0.0)
                    for g in range(nlive):
                        t = t0 + g
                        b = int(np.searchsorted(TAoff, t, side="right") - 1)
                        xp = pXT.tile([P, D * DIM], FP, tag="xp")
                        nc.vector.tensor_tensor(
                            out=xp[:].rearrange("p (d j) -> p d j", d=D),
                            in0=pw_sb[:, t * D:(t + 1) * D]
                                .rearrange("p (d o) -> p d o", o=1)
                                .to_broadcast((P, D, DIM)),
                            in1=atoms[:, g * DIM:(g + 1) * DIM]
                                .rearrange("p (o j) -> p o j", o=1)
                                .to_broadcast((P, D, DIM)),
                            op=ALU.mult)
                        xt_ps = psA.tile([P, P], FP, tag="xtp")
                        nc.tensor.transpose(
                            out=xt_ps[:], in_=xp[:], identity=identity[:])
                        xt = pXT.tile([P, P], ED, tag="xt")
                        if g % 2 == 0:
                            nc.vector.tensor_copy(out=xt[:], in_=xt_ps[:])
                        else:
                            nc.scalar.copy(out=xt[:], in_=xt_ps[:])
                        m_ps = psM.tile([P, DIM], FP, tag="mp")
                        nc.tensor.matmul(
                            out=m_ps[:], lhsT=xt[:], rhs=wb_ap(l, d, b),
                            start=True, stop=True)
                        sig = pXT.tile([P, DIM], FP, tag="sig")
                        nc.scalar.activation(
                            out=sig[:], in_=m_ps[:], func=AF.Sigmoid)
                        nc.vector.scalar_tensor_tensor(
                            out=act_slab[:, g * DIM:(g + 1) * DIM],
                            in0=m_ps[:], scalar=0.0, in1=sig[:],
                            op0=ALU.add, op1=ALU.mult)
                    nc.sync.dma_start(
                        out=act_rows[:, t0:t0 + G, :],
                        in_=act_slab[:].rearrange("p (g k) -> p g k", g=G))

        def emit_phaseB(l, d):
            m = meta["s2d"] if d == 0 else meta["d2s"]
            T_B = m["T_B"]
            live_B = m["live_B"]
            TKoff = m["TKoff"]
            sfx = f"_{d}"
            with ExitStack() as dlctx:
                io = dlctx.enter_context(
                    tc.tile_pool(name=f"ioB{l}{d}", bufs=1))
                # ---------------- phase B ----------------
                pi_sb = io.tile([P, T_B], I32)
                dloc_sb = io.tile([P, T_B], FP)
                nc.sync.dma_start(pi_sb[:], ins["piB" + sfx][:])
                nc.sync.dma_start(dloc_sb[:], ins["dloc" + sfx][:])

                pB = dlctx.enter_context(tc.tile_pool(name=f"pB{l}{d}", bufs=6))
                psB = dlctx.enter_context(
                    tc.tile_pool(name=f"psB{l}{d}", bufs=2, space="PSUM"))

                blk_of = np.searchsorted(TKoff, np.arange(T_B), side="right") - 1
                yacc = None
                for gb in range(T_B // G):
                    t0 = gb * G
                    nliveB = max(0, min(G, live_B - t0))
                    actB = pB.tile([P, G * DIM], ED, tag="actB")
                    for gg in range(nliveB):
                        nc.gpsimd.indirect_dma_start(
                            out=actB[:, gg * DIM:(gg + 1) * DIM],
                            out_offset=None, in_=act_dram[d][:],
                            in_offset=bass.IndirectOffsetOnAxis(
                                ap=pi_sb[:, t0 + gg:t0 + gg + 1], axis=0))
                    for g in range(nliveB):
                        t = t0 + g
                        kb = int(blk_of[t])
                        first = t == int(TKoff[kb])
                        last = t == min(int(TKoff[kb + 1]), live_B) - 1
                        if first:
                            yacc = psB.tile([DIM, P], FP, tag="yacc")
                        oh = pB.tile([P, P], ED, tag="oh")
                        nc.vector.tensor_scalar(
                            out=oh[:], in0=iota[:],
                            scalar1=dloc_sb[:, t:t + 1], scalar2=None,
                            op0=ALU.is_equal)
                        nc.tensor.matmul(
                            out=yacc[:], lhsT=actB[:, g * DIM:(g + 1) * DIM],
                            rhs=oh[:], start=first, stop=last)
                        if last:
                            n_hi = min((kb + 1) * P, npc[d])
                            seg = yT[d][:, kb * P:n_hi]
                            nc.vector.tensor_tensor(
                                out=seg, in0=yacc[:, :n_hi - kb * P], in1=seg,
                                op=ALU.add)

        # ---------------- layers ----------------
        for l in range(n_layers):
            emit_phaseA(l, 0)
            emit_phaseA(l, 1)
            emit_phaseB(l, 0)
            emit_phaseB(l, 1)
            if l < n_layers - 1:
                with ExitStack() as actx:
                    pg = actx.enter_context(tc.tile_pool(name=f"ag{l}", bufs=2))
                    psG = actx.enter_context(
                        tc.tile_pool(name=f"psG{l}", bufs=2, space="PSUM"))
                    for d in (0, 1):
                        nblk = nblk_of[d]
                        nfull = npc[d] // P
                        rows = pg.tile([P, nblk * DIM], FP, tag="agrows")
                        for kb in range(nblk):
                            tp = psG.tile([P, DIM], FP, tag="agT")
                            nc.tensor.transpose(
                                out=tp[:], in_=yT[d][:, kb * P:(kb + 1) * P],
                                identity=identity[:DIM, :DIM])
                            nc.vector.tensor_copy(
                                out=rows[:, kb * DIM:(kb + 1) * DIM], in_=tp[:])
                        nc.sync.dma_start(
                            out=cc_in[d][:nfull * P, :]
                                .rearrange("(t p) k -> p t k", p=P),
                            in_=rows[:, :nfull * DIM]
                                .rearrange("p (t k) -> p t k", k=DIM))
                        rem = npc[d] - nfull * P
                        if rem:
                            nc.sync.dma_start(
                                out=cc_in[d][nfull * P:, :],
                                in_=rows[:rem, nfull * DIM:(nfull + 1) * DIM])
                        nc.gpsimd.collective_compute(
                            "AllGather", ALU.bypass,
                            ins=[cc_in[d][:]],
                            outs=[y_tab[(side_of_dir[d], l)][:]],
                            replica_groups=replica_groups)

        # ---------------- readout ----------------
        with ExitStack() as rctx:
            pr = rctx.enter_context(tc.tile_pool(name="ro", bufs=2))
            psR = rctx.enter_context(
                tc.tile_pool(name="psR", bufs=2, space="PSUM"))
            CH = 512
            n_chunks = sum((npc[d] + CH - 1) // CH for d in (0, 1))
            accs = pr.tile([1, max(n_chunks, 1)], FP)
            ci = 0
            for d in (0, 1):
                for c0 in range(0, npc[d], CH):
                    c1 = min(c0 + CH, npc[d])
                    dot_ps = psR.tile([1, CH], FP, tag="dot")
                    nc.tensor.matmul(
                        out=dot_ps[:, :c1 - c0], lhsT=ro_w_sb[:],
                        rhs=yT[d][:, c0:c1], start=True, stop=True)
                    sigr = pr.tile([1, CH], FP, tag="sigr")
                    nc.scalar.activation(
                        out=sigr[:, :c1 - c0], in_=dot_ps[:, :c1 - c0],
                        func=AF.Sigmoid, bias=float(meta["ro_b"]))
                    sil = pr.tile([1, CH], FP, tag="sil")
                    nc.vector.scalar_tensor_tensor(
                        out=sil[:, :c1 - c0], in0=dot_ps[:, :c1 - c0],
                        scalar=float(meta["ro_b"]), in1=sigr[:, :c1 - c0],
                        op0=ALU.add, op1=ALU.mult,
                        accum_out=accs[:, ci:ci + 1])
                    ci += 1
            total = pr.tile([1, 1], FP)
            nc.vector.tensor_reduce(
                out=total[:], in_=accs[:, :ci], axis=mybir.AxisListType.X,
                op=ALU.add)
            nc.sync.dma_start(out=ar_in[:], in_=total[:])
            nc.gpsimd.collective_compute(
                "AllReduce", ALU.add,
                ins=[ar_in[:]], outs=[ar_out[:]],
                replica_groups=replica_groups)
            res = pr.tile([1, 1], FP)
            nc.sync.dma_start(out=res[:], in_=ar_out[:])
            nc.sync.dma_start(out=out_t[:], in_=res[:])

    nc.compile()
    return nc, ins

# ======================== runner ========================
LAST_EXEC_NS = None
EDGE_DTYPE = FP
N_CORES = 8
GBATCH = 64


def kernel(_trace=False, **inputs):
    """Full unsharded inputs -> full output (scalar f32)."""
    global LAST_EXEC_NS
    from concourse import bass_utils

    per_core, meta = prepare(inputs, n_cores=N_CORES, gbatch=GBATCH)
    shapes = {k: v.shape for k, v in per_core[0].items()}
    nc, _ = build_program(meta, shapes, edge_dtype=EDGE_DTYPE)
    in_maps = [{k: np.ascontiguousarray(v) for k, v in pc.items()}
               for pc in per_core]
    res = bass_utils.run_bass_kernel_spmd(
        nc, in_maps, core_ids=list(range(N_CORES)), trace=_trace)
    LAST_EXEC_NS = res.exec_time_ns
    return np.float32(res.results[0]["out"][0, 0])



# revision 19
# speedup vs baseline: 2.0709x; 2.0709x over previous
import sys
sys.path.insert(0, "/opt/trn_rl_repo")
"""Dimer GNN on 8 TRN2 cores — v4 (fused single-phase).

Edge-parallel sharding by owner core (dir 0: owner=dst updates y1, gathers
y0; dir 1: owner=src updates y0, gathers y1). ~50k edges per (core, dir).

Slot layout per direction (geometry COMMON across cores): tiles = columns
[P, T]; tiles are dst-block-pure (49 local blocks of 128 nodes); within a
block, edges are packed into 64-slot HALF-COLUMN strips that are bucket-pure
(strip count per (block, bucket) = max over cores -> common static geometry).
Tile t therefore has 1-2 static TP segments (partition ranges 0:64 / 64:128),
each with a static bucket.

Per direction-layer, per 4-tile quad (fused pipeline):
  atoms: l=0 from host-pregathered slab; l>=1 per-tile [128,1] indirect DMA
  row-gather from the AllGathered bf16 y_tab [N, 32].
  DVE rep-copy -> [128,(t,4rep,32j)]; 4 PE transposes -> psum atomsT_rep
  bf16 [128(d,j), 512]; DVE multiply with streamed pwrepT (u^d, channel-
  major, dead slots 0) -> xpT; per tile 1-2 MMs (lhsT=xpT segment, rhs=
  Wbflat[l,dir,bucket]) -> m [128e, 32k] f32 psum; ACT Silu -> act quad
  bf16; per tile: one-hot (DVE is_equal vs dloc) + scatter matmul
  accumulating into the block's [32,128] psum; block end: DVE add into
  channel-major yT [32, 6272] f32.
Layer end per side: PE transposes of yT -> bf16 rows -> AllGather y_tab.
Readout: chunked PE dot + ACT Silu + DVE reduce + AllReduce.
"""
import numpy as np

DIM = 32
STEP = 0.25
N_MU = 21
N_SCAL = N_MU + 1
N_LAYERS = 3
NB = 20
P = 128
D = 4
HS = 64  # strip size (half column)


def poly_coeffs():
    mu = np.linspace(0.0, 5.0, N_MU)
    C = np.zeros((NB, D, N_SCAL))
    for b in range(NB):
        rr = np.linspace(b * STEP, (b + 1) * STEP, 257)
        u = (rr - b * STEP) / STEP * 2.0 - 1.0
        V = np.stack([u ** d for d in range(D)], 1)
        G = np.exp(-0.5 * ((rr[:, None] - mu[None, :]) / STEP) ** 2)
        G = np.concatenate([G, np.ones((len(rr), 1))], 1)
        C[b], *_ = np.linalg.lstsq(V, G, rcond=None)
    return C  # [NB, D, N_SCAL]


def _ceil(a, b):
    return -(-a // b)


def shard_direction(owner, other, r, n_cores, npc, G):
    """Fused layout. Returns per-core arrays + static geometry."""
    core_of = owner // npc
    bucket = np.clip((r / STEP).astype(np.int64), 0, NB - 1)
    n_blocks = _ceil(npc, P)

    idx_by_core = [np.nonzero(core_of == c)[0] for c in range(n_cores)]
    loc_all = owner - core_of * npc
    blk_all = loc_all // P

    # common geometry: strips per (block, bucket) = max over cores
    nst = np.zeros((n_blocks, NB), np.int64)
    for c in range(n_cores):
        idx = idx_by_core[c]
        for kb in range(n_blocks):
            sel = idx[blk_all[idx] == kb]
            cnt = np.bincount(bucket[sel], minlength=NB)
            nst[kb] = np.maximum(nst[kb], _ceil(cnt, HS))
    S_kb = nst.sum(1)                       # strips per block
    tiles_kb = _ceil(S_kb, 2)               # 2 strips per tile
    Toff = np.concatenate([[0], np.cumsum(tiles_kb)]).astype(np.int64)
    T_live = int(Toff[-1])
    T = _ceil(T_live, G) * G

    # static segments: for each tile, list of (seg_lo_strip(0/1), bucket)
    # strip sigma of block kb: tile Toff[kb] + sigma//2, half sigma%2
    seg_bucket = np.full((T, 2), -1, np.int64)  # -1 = dead segment
    strip_pos = {}  # (kb, b) -> list of (tile, half) strips
    for kb in range(n_blocks):
        sigma = 0
        for b in range(NB):
            lst = []
            for _ in range(int(nst[kb, b])):
                t = int(Toff[kb] + sigma // 2)
                h = sigma % 2
                seg_bucket[t, h] = b
                lst.append((t, h))
                sigma += 1
            strip_pos[(kb, b)] = lst

    cores = []
    for c in range(n_cores):
        idx = idx_by_core[c]
        uA = np.zeros((P, T))
        vA = np.zeros((P, T))
        gidx = np.zeros((P, T), np.int32)
        dloc = np.full((P, T), 255.0)
        for kb in range(n_blocks):
            bidx = idx[blk_all[idx] == kb]
            bb = bucket[bidx]
            for b in range(NB):
                sel = bidx[bb == b]
                if len(sel) == 0:
                    continue
                sel = sel[np.argsort(other[sel], kind="stable")]
                strips = strip_pos[(kb, b)]
                for si, (t, h) in enumerate(strips):
                    seg = sel[si * HS:(si + 1) * HS]
                    if len(seg) == 0:
                        break
                    pr = h * HS + np.arange(len(seg))
                    uA[pr, t] = (r[seg] - b * STEP) / STEP * 2.0 - 1.0
                    vA[pr, t] = 1.0
                    gidx[pr, t] = other[seg]
                    dloc[pr, t] = (loc_all[seg] % P).astype(np.float64)
        cores.append(dict(uA=uA, vA=vA, gidx=gidx, dloc=dloc))

    meta = dict(T=T, T_live=T_live, Toff=Toff, seg_bucket=seg_bucket,
                n_blocks=n_blocks, tiles_kb=tiles_kb)
    return cores, meta


def prepare(inputs, n_cores=8, G=32):
    import ml_dtypes
    BF16 = ml_dtypes.bfloat16

    z0 = np.asarray(inputs["z0"]).astype(np.int64)
    z1 = np.asarray(inputs["z1"]).astype(np.int64)
    src = np.asarray(inputs["src"]).astype(np.int64)
    dst = np.asarray(inputs["dst"]).astype(np.int64)
    r = np.asarray(inputs["r"], np.float64)
    emb_w = np.asarray(inputs["emb_w"], np.float32)
    emb_b = np.asarray(inputs["emb_b"], np.float32)
    w_s2d = np.asarray(inputs["w_s2d"], np.float32)
    w_d2s = np.asarray(inputs["w_d2s"], np.float32)
    ro_w = np.asarray(inputs["ro_w"], np.float32)
    ro_b = np.asarray(inputs["ro_b"], np.float32)
    N0, N1 = len(z0), len(z1)
    npc0, npc1 = N0 // n_cores, N1 // n_cores

    C = poly_coeffs()
    scale = 1.0 / np.sqrt(N_SCAL * DIM) / np.sqrt(N0 + N1)
    wb = np.zeros((N_LAYERS, 2, NB, D * DIM, DIM), np.float32)
    for l in range(N_LAYERS):
        wb[l, 0] = np.einsum("bdi,ijk->bdjk", C, w_s2d[l]).reshape(NB, D * DIM, DIM) * scale
        wb[l, 1] = np.einsum("bdi,ijk->bdjk", C, w_d2s[l]).reshape(NB, D * DIM, DIM) * scale
    wb_sb = np.ascontiguousarray(
        wb.transpose(3, 0, 1, 2, 4).reshape(D * DIM, N_LAYERS * 2 * NB * DIM)
    ).astype(BF16)

    emb_eff = (emb_w + emb_b[None, :]).astype(np.float32)
    emb_b16 = emb_eff.astype(BF16)

    s2d_cores, s2d_meta = shard_direction(dst, src, r, n_cores, npc1, G)
    d2s_cores, d2s_meta = shard_direction(src, dst, r, n_cores, npc0, G)
    metas = {0: s2d_meta, 1: d2s_meta}
    z_of_side = {0: z0, 1: z1}
    gside_of_dir = {0: 0, 1: 1}
    npc_of_side = {0: npc0, 1: npc1}

    per_core = []
    for c in range(n_cores):
        dd = {}
        for d, cores in ((0, s2d_cores), (1, d2s_cores)):
            cd = cores[c]
            T = metas[d]["T"]
            u, v = cd["uA"], cd["vA"]
            pows = np.stack([v * (u ** dg) for dg in range(D)], 0)
            pw = np.repeat(pows, DIM, axis=0)  # [128, P, T]
            pwrepT = np.ascontiguousarray(
                pw.transpose(0, 2, 1).reshape(D * DIM, T * P)).astype(BF16)
            zg = z_of_side[gside_of_dir[d]]
            at0 = emb_b16[zg[cd["gidx"]]].astype(np.float32).reshape(P, T * DIM)
            at0 = (at0 * cd["vA"].repeat(DIM, axis=1)).astype(BF16)
            sfx = f"_{d}"
            dd["pwrepT" + sfx] = pwrepT
            dd["atoms0" + sfx] = np.ascontiguousarray(at0)
            dd["gidx" + sfx] = np.ascontiguousarray(cd["gidx"])
            dd["dloc" + sfx] = np.ascontiguousarray(cd["dloc"].astype(np.float32))
        for s in (0, 1):
            npc = npc_of_side[s]
            nblk = _ceil(npc, P)
            zz = np.zeros(nblk * P, np.int64)
            zz[:npc] = z_of_side[s][c * npc:(c + 1) * npc]
            yT0 = emb_eff[zz].T.copy()
            yT0[:, npc:] = 0.0
            dd[f"yT0_{s}"] = np.ascontiguousarray(yT0)  # [32, nblk*P] f32
            dd[f"ytab0_{s}"] = emb_b16[z_of_side[s]]     # [N, 32] bf16
        dd["wb_sb"] = wb_sb
        dd["ro_w"] = ro_w
        per_core.append(dd)

    meta = dict(m0=s2d_meta, m1=d2s_meta, n_cores=n_cores, G=G,
                npc0=npc0, npc1=npc1, ro_b=float(ro_b[0]),
                N_of_side={0: N0, 1: N1})
    return per_core, meta


# ======================== bass program ========================
from contextlib import ExitStack

from concourse import bass, mybir
from concourse import bacc
import concourse.tile as tile
from concourse.masks import make_identity

FP = mybir.dt.float32
BF = mybir.dt.bfloat16
I32 = mybir.dt.int32
AF = mybir.ActivationFunctionType
ALU = mybir.AluOpType
QUAD = 4


def build_program(meta, shapes, n_layers=3):
    n_cores = meta["n_cores"]
    G = meta["G"]
    npc = {0: meta["npc0"], 1: meta["npc1"]}
    N_side = meta["N_of_side"]
    metas = {0: meta["m0"], 1: meta["m1"]}
    side_of_dir = {0: 1, 1: 0}
    gside_of_dir = {0: 0, 1: 1}

    nc = bacc.Bacc("TRN2", target_bir_lowering=False, debug=False,
                   num_devices=n_cores)

    ins = {}
    for name, shp in shapes.items():
        if name.startswith("gidx"):
            dt = I32
        elif name.startswith(("yT0", "ro_w", "dloc")):
            dt = FP
        else:
            dt = BF
        ins[name] = nc.dram_tensor(name, list(shp), dt, kind="ExternalInput")

    out_t = nc.dram_tensor("out", [1, 1], FP, kind="ExternalOutput")

    cc_in = {s: nc.dram_tensor(f"cc_in_{s}", [npc[s], DIM], BF)
             for s in (0, 1)}
    y_tab = {}
    for s in (0, 1):
        for l in (1, 2):
            y_tab[(s, l)] = nc.dram_tensor(
                f"y_tab_{s}_{l}", [N_side[s], DIM], BF, addr_space="Shared")
    ar_in = nc.dram_tensor("ar_in", [1, 1], FP)
    ar_out = nc.dram_tensor("ar_out", [1, 1], FP, addr_space="Shared")

    replica_groups = [list(range(n_cores))]
    WBCOLS = n_layers * 2 * NB * DIM
    nblk_of = {s: _ceil(npc[s], P) for s in (0, 1)}

    with tile.TileContext(nc) as tc, ExitStack() as ctx:
        const = ctx.enter_context(tc.tile_pool(name="const", bufs=1))
        identity = const.tile([P, P], FP)
        make_identity(nc, identity[:])
        ident_b = const.tile([P, P], BF)
        nc.vector.tensor_copy(out=ident_b[:], in_=identity[:])
        iota_i = const.tile([P, P], I32)
        nc.gpsimd.iota(iota_i[:], [[1, P]], base=0, channel_multiplier=0)
        iota_b = const.tile([P, P], BF)
        nc.vector.tensor_copy(out=iota_b[:], in_=iota_i[:])
        ro_w_sb = const.tile([DIM, 1], FP)
        nc.sync.dma_start(ro_w_sb[:], ins["ro_w"][:])
        wb_sb = const.tile([P, WBCOLS], BF)
        nc.sync.dma_start(wb_sb[:], ins["wb_sb"][:])

        def wb_ap(l, d, b):
            off = ((l * 2 + d) * NB + b) * DIM
            return wb_sb[:, off:off + DIM]

        emeta = ctx.enter_context(tc.tile_pool(name="emeta", bufs=1))
        gidx_sb, dloc_sb = {}, {}
        for d in (0, 1):
            T = metas[d]["T"]
            gidx_sb[d] = emeta.tile([P, T], I32, name=f"gidx{d}", tag=f"gidx{d}")
            nc.sync.dma_start(gidx_sb[d][:], ins[f"gidx_{d}"][:])
            dloc_sb[d] = emeta.tile([P, T], FP, name=f"dloc{d}", tag=f"dloc{d}")
            nc.sync.dma_start(dloc_sb[d][:], ins[f"dloc_{d}"][:])

        yT = {}
        for s in (0, 1):
            yT[s] = const.tile([DIM, nblk_of[s] * P], FP, name=f"yT{s}", tag=f"yT{s}")
            nc.sync.dma_start(yT[s][:], ins[f"yT0_{s}"][:])

        # ---------------- fused direction-layer ----------------
        def emit_dirlayer(l, d):
            m = metas[d]
            T, T_live = m["T"], m["T_live"]
            Toff = m["Toff"]
            seg_bucket = m["seg_bucket"]
            n_blocks = m["n_blocks"]
            s = side_of_dir[d]
            gs = gside_of_dir[d]
            # block of tile, and first/last live tile per block
            blk_of = np.searchsorted(Toff, np.arange(T), side="right") - 1
            with ExitStack() as actx:
                pA = actx.enter_context(tc.tile_pool(name=f"pA{l}{d}", bufs=2))
                pX = actx.enter_context(tc.tile_pool(name=f"pX{l}{d}", bufs=3))
                pO = actx.enter_context(tc.tile_pool(name=f"pO{l}{d}", bufs=4))
                psT = actx.enter_context(
                    tc.tile_pool(name=f"psT{l}{d}", bufs=2, space="PSUM"))
                psM = actx.enter_context(
                    tc.tile_pool(name=f"psM{l}{d}", bufs=2, space="PSUM"))
                psB = actx.enter_context(
                    tc.tile_pool(name=f"psB{l}{d}", bufs=2, space="PSUM"))
                yacc = None
                nG = T // G
                for g in range(nG):
                    t0 = g * G
                    nlive = max(0, min(G, T_live - t0))
                    if nlive == 0:
                        continue
                    atoms = pA.tile([P, G * DIM], BF, tag="atoms")
                    if l == 0:
                        nc.sync.dma_start(
                            atoms[:],
                            ins[f"atoms0_{d}"][:, t0 * DIM:(t0 + G) * DIM])
                    else:
                        tab = y_tab[(gs, l)]
                        for tt in range(nlive):
                            nc.gpsimd.indirect_dma_start(
                                out=atoms[:, tt * DIM:(tt + 1) * DIM],
                                out_offset=None, in_=tab[:],
                                in_offset=bass.IndirectOffsetOnAxis(
                                    ap=gidx_sb[d][:, t0 + tt:t0 + tt + 1],
                                    axis=0))
                    pw = pA.tile([P, G * P], BF, tag="pw")
                    nc.sync.dma_start(
                        pw[:], ins[f"pwrepT_{d}"][:, t0 * P:(t0 + G) * P])
                    nq = _ceil(nlive, QUAD)
                    for q in range(nq):
                        qt = q * QUAD
                        repq = pX.tile([P, QUAD * P], BF, tag="repq")
                        nc.vector.tensor_copy(
                            out=repq[:].rearrange(
                                "p (t r j) -> p t r j", r=QUAD, j=DIM),
                            in_=atoms[:, qt * DIM:(qt + QUAD) * DIM]
                            .rearrange("p (t o j) -> p t o j", o=1, j=DIM)
                            .to_broadcast((P, QUAD, QUAD, DIM)))
                        tps = psT.tile([P, QUAD * P], BF, tag="tps")
                        for cq in range(QUAD):
                            nc.tensor.transpose(
                                out=tps[:, cq * P:(cq + 1) * P],
                                in_=repq[:, cq * P:(cq + 1) * P],
                                identity=ident_b[:])
                        xpt = pX.tile([P, QUAD * P], BF, tag="xpt")
                        nc.vector.tensor_tensor(
                            out=xpt[:], in0=tps[:],
                            in1=pw[:, qt * P:(qt + QUAD) * P], op=ALU.mult)
                        mq = psM.tile([P, QUAD * DIM], FP, tag="mq")
                        if g == 0 and q < 2:
                            nc.vector.memset(mq[:], 0.0)
                        for cq in range(QUAD):
                            t = t0 + qt + cq
                            if t >= T_live:
                                continue
                            sb0, sb1 = seg_bucket[t]
                            col = xpt[:, cq * P:(cq + 1) * P]
                            if sb0 >= 0 and sb0 == sb1:
                                nc.tensor.matmul(
                                    out=mq[:, cq * DIM:(cq + 1) * DIM],
                                    lhsT=col, rhs=wb_ap(l, d, sb0),
                                    start=True, stop=True)
                            else:
                                if sb0 >= 0:
                                    nc.tensor.matmul(
                                        out=mq[:HS, cq * DIM:(cq + 1) * DIM],
                                        lhsT=col[:, :HS],
                                        rhs=wb_ap(l, d, sb0),
                                        start=True, stop=True,
                                        tile_position=(0, 0))
                                if sb1 >= 0:
                                    nc.tensor.matmul(
                                        out=mq[HS:, cq * DIM:(cq + 1) * DIM],
                                        lhsT=col[:, HS:],
                                        rhs=wb_ap(l, d, sb1),
                                        start=True, stop=True,
                                        tile_position=(0, HS))
                        act4 = pX.tile([P, QUAD * DIM], BF, tag="act4")
                        nc.scalar.activation(
                            out=act4[:], in_=mq[:], func=AF.Silu)
                        # scatter the quad's tiles
                        for cq in range(QUAD):
                            t = t0 + qt + cq
                            if t >= T_live:
                                continue
                            kb = int(blk_of[t])
                            first = t == int(Toff[kb])
                            last = t == min(int(Toff[kb + 1]), T_live) - 1
                            oh = pO.tile([P, P], BF, tag="oh")
                            nc.vector.tensor_scalar(
                                out=oh[:], in0=iota_b[:],
                                scalar1=dloc_sb[d][:, t:t + 1], scalar2=None,
                                op0=ALU.is_equal)
                            if first:
                                yacc = psB.tile([DIM, P], FP, tag="yacc")
                            nc.tensor.matmul(
                                out=yacc[:],
                                lhsT=act4[:, cq * DIM:(cq + 1) * DIM],
                                rhs=oh[:], start=first, stop=last)
                            if last:
                                n_hi = min((kb + 1) * P, npc[s])
                                seg = yT[s][:, kb * P:n_hi]
                                nc.vector.tensor_tensor(
                                    out=seg, in0=yacc[:, :n_hi - kb * P],
                                    in1=seg, op=ALU.add)

            # AllGather updated side for next layer
            if l < n_layers - 1:
                nblk = nblk_of[s]
                nfull = npc[s] // P
                with ExitStack() as gctx:
                    pg = gctx.enter_context(
                        tc.tile_pool(name=f"ag{l}{s}", bufs=2))
                    psG = gctx.enter_context(
                        tc.tile_pool(name=f"psG{l}{s}", bufs=4, space="PSUM"))
                    rows = pg.tile([P, nblk * DIM], BF, tag="agrows")
                    for kb in range(nblk):
                        tp = psG.tile([P, DIM], FP, tag="agT")
                        nc.tensor.transpose(
                            out=tp[:], in_=yT[s][:, kb * P:(kb + 1) * P],
                            identity=identity[:DIM, :DIM])
                        if kb % 2 == 0:
                            nc.vector.tensor_copy(
                                out=rows[:, kb * DIM:(kb + 1) * DIM], in_=tp[:])
                        else:
                            nc.scalar.copy(
                                out=rows[:, kb * DIM:(kb + 1) * DIM], in_=tp[:])
                    nc.sync.dma_start(
                        out=cc_in[s][:nfull * P, :]
                        .rearrange("(t p) k -> p t k", p=P),
                        in_=rows[:, :nfull * DIM]
                        .rearrange("p (t k) -> p t k", k=DIM))
                    rem = npc[s] - nfull * P
                    if rem:
                        nc.sync.dma_start(
                            out=cc_in[s][nfull * P:, :],
                            in_=rows[:rem, nfull * DIM:(nfull + 1) * DIM])
                    nc.gpsimd.collective_compute(
                        "AllGather", ALU.bypass,
                        ins=[cc_in[s][:]],
                        outs=[y_tab[(s, l + 1)][:]],
                        replica_groups=replica_groups)

        for l in range(n_layers):
            dirs = (0, 1) if l % 2 == 0 else (1, 0)
            for d in dirs:
                emit_dirlayer(l, d)

        # ---------------- readout ----------------
        with ExitStack() as rctx:
            pr = rctx.enter_context(tc.tile_pool(name="ro", bufs=2))
            psR = rctx.enter_context(
                tc.tile_pool(name="psR", bufs=2, space="PSUM"))
            CH = 512
            n_chunks = sum(_ceil(npc[s], CH) for s in (0, 1))
            accs = pr.tile([1, max(n_chunks, 1)], FP)
            ci = 0
            for s in (0, 1):
                for c0 in range(0, npc[s], CH):
                    c1 = min(c0 + CH, npc[s])
                    dot_ps = psR.tile([1, CH], FP, tag="dot")
                    nc.tensor.matmul(
                        out=dot_ps[:, :c1 - c0], lhsT=ro_w_sb[:],
                        rhs=yT[s][:, c0:c1], start=True, stop=True)
                    sil = pr.tile([1, CH], FP, tag="sil")
                    nc.scalar.activation(
                        out=sil[:, :c1 - c0], in_=dot_ps[:, :c1 - c0],
                        func=AF.Silu, bias=float(meta["ro_b"]))
                    nc.vector.tensor_reduce(
                        out=accs[:, ci:ci + 1], in_=sil[:, :c1 - c0],
                        axis=mybir.AxisListType.X, op=ALU.add)
                    ci += 1
            total = pr.tile([1, 1], FP)
            nc.vector.tensor_reduce(
                out=total[:], in_=accs[:, :ci], axis=mybir.AxisListType.X,
                op=ALU.add)
            nc.sync.dma_start(out=ar_in[:], in_=total[:])
            nc.gpsimd.collective_compute(
                "AllReduce", ALU.add,
                ins=[ar_in[:]], outs=[ar_out[:]],
                replica_groups=replica_groups)
            res = pr.tile([1, 1], FP)
            nc.sync.dma_start(out=res[:], in_=ar_out[:])
            nc.sync.dma_start(out=out_t[:], in_=res[:])

    nc.compile()
    return nc


# ======================== runner ========================
LAST_EXEC_NS = None
N_CORES = 8
GBATCH = 32


def kernel(_trace=False, **inputs):
    global LAST_EXEC_NS
    from concourse import bass_utils

    per_core, meta = prepare(inputs, n_cores=N_CORES, G=GBATCH)
    shapes = {k: v.shape for k, v in per_core[0].items()}
    nc = build_program(meta, shapes)
    in_maps = [{k: np.ascontiguousarray(v) for k, v in pc.items()}
               for pc in per_core]
    res = bass_utils.run_bass_kernel_spmd(
        nc, in_maps, core_ids=list(range(N_CORES)), trace=_trace)
    LAST_EXEC_NS = res.exec_time_ns
    return np.float32(res.results[0]["out"][0, 0])
